# revision 1
# baseline (speedup 1.0000x reference)
"""Causal multi-head self-attention on 8 trn2 NeuronCores.

Sharding: 8 cores = 4 batch x 2 head-groups. Core i handles batch i//2 and
heads (i%2)*8 .. (i%2)*8+8 (8 of 16 heads, 512 of 1024 d_model columns).
Each core computes a full (2048, 1024) partial output (its head group pushed
through its w_proj row-slice); the host sums the two partials per batch
element (the tensor-parallel all-reduce done host-side).

Per-core dataflow (everything in transposed layout to avoid transposing the
big attention intermediates):
  x^T        : PE-transpose of x tiles (128 transposes of [128,128])
  Q^T, K^T   : w_q/w_k stationary, x^T moving  -> [cols, seq] layout
  V          : x^T stationary, w_v moving      -> natural [seq, cols] layout,
               stored with a ones-column per head (V_aug [128, 65]) so the
               softmax denominator rides along the AV matmul as output row 64
  S^T        : K^T stationary, Q^T moving, two heads packed into the 128 PE
               rows (contraction = head_dim 64, partitions 0-63 / 64-127)
  P^T        : exp(S^T * rsqrt(hd)) on ScalarE (no max-subtraction needed:
               |S|*rsqrt stays < ~10), causal handled by skipping k>q chunks,
               zeroing the invalid prefix, and a 0/1 upper-tri mask multiply
               on the diagonal 128x128 block
  O^T_aug    : V_aug stationary, P^T moving, accumulated over k-tiles in PSUM
  O^T        : O^T_aug rows 0-63 * (1/row 64); 1/d = exp(-ln d) batched on
               ScalarE, broadcast over 64 partitions via a K=1 outer-product
               matmul (DVE reciprocal is 8 cyc/elem and partition-serial)
  out        : O^T stationary, w_proj rows moving -> natural [seq, 1024]

Linear layers (QKV, V, proj, x^T) run in float32r (full-rate fp32 on the PE
for moving dim >= 256; plain fp32 is 4 cyc/row). Attention (Q^T/K^T/P^T/V)
runs in bf16: fp32r matmuls with K=64 or M=65 drop to ~half rate, bf16 keeps
full rate and lets the paired S^T matmuls overlap via PE row groups.

Schedule: one software-pipelined stream — S^T groups of 4 with exp chewing
behind, AV groups trailing one group (group order pinned with dep edges;
per-instruction alternation of the two shapes halves the PE rate), the next
pair's QKV-projection chunks + the V tail + deferred softmax-denominator
normalization interleaved as PE filler at group boundaries.
"""

import numpy as np

import concourse.bass as bass
import concourse.mybir as mybir
import concourse.tile as tile
from concourse import bacc
from concourse.bass_utils import run_bass_kernel_spmd
from concourse.masks import make_identity, make_upper_triangular
from concourse.tile_rust import add_dep_helper

F32 = mybir.dt.float32
F32R = mybir.dt.float32r
BF16 = mybir.dt.bfloat16
AF = mybir.ActivationFunctionType

SEQ = 2048
DM = 1024
COLS = 512          # head-cols per core (8 heads x 64)
HD = 64
P = 128
N_CORES = 8
RSQRT = 0.125       # 1/sqrt(64)

SEQ_T = SEQ // P    # 16 seq tiles
DM_T = DM // P      # 8 d_model tiles
QC = 512            # q-chunk (PSUM free size)
N_QC = SEQ // QC    # 4 q chunks
KT_PER_QC = QC // P  # 4 k-tiles per q chunk


def _build_core_program():
    nc = bacc.Bacc(
        "TRN2", target_bir_lowering=False, debug=False, num_devices=N_CORES
    )
    x = nc.dram_tensor("x", [SEQ, DM], F32, kind="ExternalInput").ap()
    wq = nc.dram_tensor("wq", [DM, COLS], F32, kind="ExternalInput").ap()
    wk = nc.dram_tensor("wk", [DM, COLS], F32, kind="ExternalInput").ap()
    wv = nc.dram_tensor("wv", [DM, COLS], F32, kind="ExternalInput").ap()
    wp = nc.dram_tensor("wp", [COLS, DM], F32, kind="ExternalInput").ap()
    out = nc.dram_tensor("out", [SEQ, DM], F32, kind="ExternalOutput").ap()

    with tile.TileContext(nc) as tc:
        _emit(tc, x, wq, wk, wv, wp, out)
    nc.compile()
    return nc


def _emit(tc, x, wq, wk, wv, wp, out):
    nc = tc.nc

    # --- program-lifetime pools -------------------------------------------
    const_pool = tc.alloc_tile_pool(name="const", bufs=1)
    psum_mm = tc.alloc_tile_pool(name="psum_mm", bufs=4, space="PSUM")
    psum_acc = tc.alloc_tile_pool(name="psum_acc", bufs=4, space="PSUM")

    ident = const_pool.tile([P, P], F32, tag="ident")
    make_identity(nc, ident[:])
    mask01 = const_pool.tile([P, P], BF16, tag="mask01")
    # 1.0 where free-idx (q) >= partition-idx (k), else 0 — causal in S^T layout
    make_upper_triangular(nc, mask01[:], val=1.0, diag=True)
    # fp32r constants must be produced by a rounding engine write, not memset
    cstage = const_pool.tile([P, 3 * P], F32, tag="cstage")
    nc.vector.memset(cstage[:], 0.0)
    zeros384 = const_pool.tile([P, 3 * P], BF16, tag="zeros384")
    nc.vector.tensor_copy(zeros384[:], cstage[:])
    nc.vector.memset(cstage[:, 0:HD], 1.0)
    ones_bc = const_pool.tile([P, HD], F32R, tag="ones_bc")
    nc.vector.tensor_copy(ones_bc[:], cstage[:, 0:HD])

    # --- long-lived intermediates -----------------------------------------
    xt_pool = tc.alloc_tile_pool(name="xt", bufs=1)
    xt = xt_pool.tile([P, DM_T * SEQ], F32R, tag="xt")  # [128, 8*2048], x^T

    # ===== phase A: load x, build x^T =====================================
    # 4 PE transposes share one PSUM bank; one strided DVE copy drains all 4
    xt_v = xt[:].rearrange("p (d s) -> p d s", d=DM_T)
    xload_pool = tc.alloc_tile_pool(name="xload", bufs=3)
    for s in range(SEQ_T):
        xin = xload_pool.tile([P, DM], F32, tag="xin")
        nc.sync.dma_start(xin[:], x[s * P : (s + 1) * P, :])
        for d4 in range(DM_T // 4):
            pt = psum_mm.tile([P, QC], F32, tag="mm")
            for j in range(4):
                d = d4 * 4 + j
                nc.tensor.transpose(
                    pt[:, j * P : (j + 1) * P],
                    xin[:, d * P : (d + 1) * P],
                    ident[:],
                )
            nc.vector.tensor_copy(
                xt_v[:, d4 * 4 : d4 * 4 + 4, s * P : (s + 1) * P],
                pt[:].rearrange("p (j q) -> p j q", j=4),
            )
    xload_pool.release()

    # ===== phase C: V (natural layout) + ones columns =====================
    oT_pool = tc.alloc_tile_pool(name="oT", bufs=1)
    oT = oT_pool.tile([P, 4 * SEQ], F32R, tag="oT")  # 4 head-pair tiles
    dcol_pool = tc.alloc_tile_pool(name="dcol", bufs=2)  # outlives D (E tail)

    vaug_pool = tc.alloc_tile_pool(name="vaug", bufs=1)
    # V in natural [seq, cols] layout + per-head ones column (softmax denom
    # rides the AV matmul as output row 64): seq-tile-major, 8 x (64 V + 1)
    vaug = vaug_pool.tile([P, SEQ_T * 8 * (HD + 1)], BF16, tag="vaug")
    vaug_v = vaug[:].rearrange("p (s h e) -> p s h e", s=SEQ_T, h=8, e=HD + 1)

    wv_pool = tc.alloc_tile_pool(name="wv", bufs=1)
    wv_sb = wv_pool.tile([P, DM_T * COLS], F32R, tag="wv_sb")
    wstg2_pool = tc.alloc_tile_pool(name="wstg2", bufs=3)
    for d in range(DM_T):
        wst = wstg2_pool.tile([P, COLS], F32, tag="wst2")
        nc.sync.dma_start(wst[:], wv[d * P : (d + 1) * P, :])
        nc.vector.tensor_copy(wv_sb[:, d * COLS : (d + 1) * COLS], wst[:])
    wstg2_pool.release()
    def v_chunk(s):
        ps = psum_mm.tile([P, QC], F32, tag="mm", name=f"vps_{s}")
        for d in range(DM_T):
            nc.tensor.matmul(
                ps[:],
                (xt[:, d * SEQ + s * P : d * SEQ + (s + 1) * P]),
                (wv_sb[:, d * COLS : (d + 1) * COLS]),
                start=(d == 0),
                stop=(d == DM_T - 1),
            )
        nc.vector.tensor_copy(
            vaug_v[:, s, :, 0:HD],
            ps[:].rearrange("p (h e) -> p h e", h=8),
        )
        nc.vector.tensor_copy(
            vaug_v[:, s, :, HD : HD + 1],
            cstage[:, 0:8].rearrange("p (a b) -> p a b", b=1),
        )

    # first half upfront (needed by pair 0's early AVs); the rest is
    # interleaved into pair 0's attention stream as PE filler
    for s in range(SEQ_T // 2):
        v_chunk(s)
    v_rest = iter(range(SEQ_T // 2, SEQ_T))

    # ===== phase D: attention =============================================
    qkpair_pool = tc.alloc_tile_pool(name="qkpair", bufs=2)
    pt_pool = tc.alloc_tile_pool(name="ptile", bufs=12)
    wpair_pool = tc.alloc_tile_pool(name="wpair", bufs=2)
    wstgd_pool = tc.alloc_tile_pool(name="wstgd", bufs=3)

    # Q^T/K^T for one head pair, computed just-in-time into SBUF. Yields
    # after each PSUM-chunk so the caller can interleave these matmuls into
    # the previous pair's attention stream (fills PE while ScalarE crunches
    # exp). w_q/w_k column slices are streamed per pair with a rounding copy
    # (DMA cannot produce fp32r).
    def make_qk_pair(hp):
        qTn = qkpair_pool.tile([P, SEQ], BF16, tag="qTp", name=f"qTp_{hp}")
        kTn = qkpair_pool.tile([P, SEQ], BF16, tag="kTp", name=f"kTp_{hp}")
        wpq = wpair_pool.tile([P, DM_T * P], F32R, tag="wpq", name=f"wpq_{hp}")
        wpk = wpair_pool.tile([P, DM_T * P], F32R, tag="wpk", name=f"wpk_{hp}")

        def chunks():
            for w_dram, wsb in ((wq, wpq), (wk, wpk)):
                for d in range(DM_T):
                    wst = wstgd_pool.tile([P, P], F32, tag="wst")
                    nc.sync.dma_start(
                        wst[:],
                        w_dram[d * P : (d + 1) * P, hp * P : (hp + 1) * P],
                    )
                    nc.vector.tensor_copy(wsb[:, d * P : (d + 1) * P], wst[:])
            yield None
            for wsb, dstT in ((wpq, qTn), (wpk, kTn)):
                for n in range(N_QC):
                    ps = psum_mm.tile([P, QC], F32, tag="mm")
                    for d in range(DM_T):
                        nc.tensor.matmul(
                            ps[:],
                            wsb[:, d * P : (d + 1) * P],
                            xt[:, d * SEQ + n * QC : d * SEQ + (n + 1) * QC],
                            start=(d == 0),
                            stop=(d == DM_T - 1),
                        )
                    nc.vector.tensor_copy(
                        dstT[:, n * QC : (n + 1) * QC], ps[:]
                    )
                    yield None

        return qTn, kTn, chunks()

    def normalize_gen(jobs, dcol, rinv_name):
        # batched 1/denom via exp(-ln d) on ScalarE, computed in place on the
        # collector (DVE reciprocal is 8 cyc/elem; batching keeps act-table
        # reloads to ~3 per batch). Collector rows sit at partition bases
        # {0,32,64,96} x free slots. Yields so the caller can interleave the
        # work into later attention groups instead of stalling the PE.
        slots = sorted({idx // 4 for idx, _ in jobs})
        f0, f1 = min(slots) * QC, (max(slots) + 1) * QC
        for b in range(4):
            nc.scalar.activation(
                dcol[32 * b : 32 * b + 1, f0:f1],
                dcol[32 * b : 32 * b + 1, f0:f1],
                AF.Ln,
            )
        yield None
        for b in range(4):
            nc.scalar.activation(
                dcol[32 * b : 32 * b + 1, f0:f1],
                dcol[32 * b : 32 * b + 1, f0:f1],
                AF.Exp, scale=-1.0,
            )
        yield None
        for j, (idx, dst) in enumerate(jobs):
            b, slot = 32 * (idx % 4), idx // 4
            # broadcast over 64 partitions as a K=1 outer-product matmul
            rc = psum_mm.tile(
                [P, QC], F32, tag="mm", name=f"rc_{rinv_name}_{idx}"
            )
            nc.tensor.matmul(
                rc[0:HD, :],
                ones_bc[b : b + 1, :],
                dcol[b : b + 1, slot * QC : (slot + 1) * QC],
                start=True,
                stop=True,
                tile_position=(b, 0),
            )
            nc.vector.tensor_mul(dst, dst, rc[0:HD, :])
            if j % 2 == 1:
                yield None

    # Software pipeline: S^T jobs stream in groups of G; each group's AV
    # jobs trail G jobs behind, so ScalarE (exp) always has fresh S psums
    # while the PE alternates between same-shape bursts. Group order on the
    # PE is pinned with dep edges — the scheduler would otherwise weave the
    # two shapes instruction-by-instruction, which halves the matmul rate.
    GRP = 4
    norm_jobs = []
    dcol = None
    pending = []  # AV jobs waiting: (po, hh, kt, nkt, h, pt, block_done, ...)
    prev_group_last = [None]

    def emit_av_group(n):
        first = True
        last = None
        for _ in range(n):
            job = pending.pop(0)
            av = nc.tensor.matmul(
                job["po"][0 : HD + 1, :],
                vaug_v[:, job["kt"], job["h"], :],
                job["pt"][:],
                start=(job["kt"] == 0),
                stop=(job["kt"] == job["nkt"] - 1),
            )
            if first and prev_group_last[0] is not None:
                add_dep_helper(
                    av.ins, prev_group_last[0], sync=False,
                    reason="pin AV group after S group",
                )
                first = False
            last = av
            if job["finish"] is not None:
                job["finish"]()
        if last is not None:
            prev_group_last[0] = last.ins

    cur = make_qk_pair(0)
    for _ in cur[2]:  # pair 0 computed upfront
        pass
    nxt = None
    grp_count = 0
    filler_gens = []  # deferred normalize work, advanced at group boundaries
    for hp in range(4):  # head pair
        if hp % 2 == 0:
            dcol = dcol_pool.tile(
                [P, 4 * QC], F32R, tag="dcol", name=f"dcol_{hp}"
            )
        qT, kT = cur[0], cur[1]
        if hp < 3:
            nxt = make_qk_pair(hp + 1)
        for qc in range(N_QC):
            po = {}
            for hh in range(2):
                po[hh] = psum_acc.tile(
                    [P, QC], F32, tag="po", name=f"po_{hp}_{qc}_{hh}"
                )
            nkt = KT_PER_QC * qc + KT_PER_QC
            sgrp = 0
            for kt in range(nkt):
                for hh in range(2):
                    base = hh * HD
                    ps_s = psum_mm.tile([P, QC], F32, tag="mm")
                    s_mm = nc.tensor.matmul(
                        ps_s[:],
                        (kT[base : base + HD, kt * P : (kt + 1) * P]),
                        (qT[base : base + HD, qc * QC : (qc + 1) * QC]),
                        start=True,
                        stop=True,
                    )
                    if sgrp == 0 and prev_group_last[0] is not None:
                        add_dep_helper(
                            s_mm.ins, prev_group_last[0], sync=False,
                            reason="pin S group after AV group",
                        )
                    sgrp += 1
                    pt = pt_pool.tile(
                        [P, QC], BF16, tag="pt", name=f"pt_{kt}_{hh}"
                    )
                    if kt // KT_PER_QC == qc:
                        off = (kt - KT_PER_QC * qc) * P
                        if off > 0:
                            nc.vector.tensor_copy(
                                pt[:, 0:off], zeros384[:, 0:off]
                            )
                        nc.scalar.activation(
                            pt[:, off:QC], ps_s[:, off:QC], AF.Exp, scale=RSQRT
                        )
                        nc.vector.tensor_mul(
                            pt[:, off : off + P], pt[:, off : off + P], mask01[:]
                        )
                    else:
                        nc.scalar.activation(pt[:], ps_s[:], AF.Exp, scale=RSQRT)

                    finish = None
                    if kt == nkt - 1:
                        # after this head's last AV: drain denom + O^T copy
                        def finish(hp=hp, qc=qc, hh=hh, po=po):
                            idx = (hp % 2) * 8 + qc * 2 + hh
                            b, slot = 32 * (idx % 4), idx // 4
                            nc.vector.tensor_copy(
                                dcol[b : b + 1, slot * QC : (slot + 1) * QC],
                                po[hh][HD : HD + 1, :],
                            )
                            dst = oT[
                                hh * HD : (hh + 1) * HD,
                                hp * SEQ + qc * QC : hp * SEQ + (qc + 1) * QC,
                            ]
                            nc.vector.tensor_copy(dst, po[hh][0:HD, :])
                            norm_jobs.append((idx, dst))

                    pending.append(
                        dict(po=po[hh], kt=kt, nkt=nkt, h=2 * hp + hh,
                             pt=pt, finish=finish)
                    )
                    if sgrp == GRP:
                        prev_group_last[0] = s_mm.ins
                        if len(pending) > GRP:
                            emit_av_group(GRP)
                        grp_count += 1
                        if grp_count % 2 == 0:
                            if nxt is not None:
                                next(nxt[2], None)  # next pair's QK chunks
                        elif filler_gens:
                            if next(filler_gens[0], True) is True:
                                filler_gens.pop(0)
                        else:
                            s_next = next(v_rest, None)
                            if s_next is not None:
                                v_chunk(s_next)  # interleave V tail
                        sgrp = 0
        emit_av_group(len(pending))  # drain this pair's AVs
        if hp >= 1:
            # hp0+hp1 together; hp2 and hp3 as their own batches so only
            # hp3's eight jobs remain in the end-of-kernel tail
            if hp == 1 or hp == 2 or hp == 3:
                gen = normalize_gen(norm_jobs, dcol, f"b{hp}")
                norm_jobs = []
                if hp == 3:
                    norm_tail_gen = gen  # drained in phase E, qc by qc
                else:
                    filler_gens.append(gen)
        if nxt is not None:
            for _ in nxt[2]:  # finish any remaining QK chunks
                pass
            cur = nxt
            nxt = None
    for g in filler_gens:  # safety drain
        for _ in g:
            pass
    wstgd_pool.release()
    wpair_pool.release()
    pt_pool.release()
    qkpair_pool.release()
    wv_pool.release()
    vaug_pool.release()

    # ===== phase E: projection ============================================
    # hp3's normalization drains qc-by-qc with the dependent proj s-tiles
    # chained right behind each step (no serial normalize tail)
    wp_pool = tc.alloc_tile_pool(name="wp", bufs=1)
    wp_sb = wp_pool.tile([P, 4 * DM], F32R, tag="wp_sb")
    wstg3_pool = tc.alloc_tile_pool(name="wstg3", bufs=2)
    for c in range(4):
        wst = wstg3_pool.tile([P, DM], F32, tag="wst3")
        nc.sync.dma_start(wst[:], wp[c * P : (c + 1) * P, :])
        nc.vector.tensor_copy(wp_sb[:, c * DM : (c + 1) * DM], wst[:])
    wstg3_pool.release()
    ostage_pool = tc.alloc_tile_pool(name="ostage", bufs=3)

    def proj_chunk(s):
        ost = ostage_pool.tile([P, DM], F32, tag="ost", name=f"ost_{s}")
        for n2 in range(2):
            ps = psum_mm.tile([P, QC], F32, tag="mm")
            for c in range(4):
                nc.tensor.matmul(
                    ps[:],
                    (oT[:, c * SEQ + s * P : c * SEQ + (s + 1) * P]),
                    (wp_sb[:, c * DM + n2 * QC : c * DM + (n2 + 1) * QC]),
                    start=(c == 0),
                    stop=(c == 3),
                )
            nc.vector.tensor_copy(ost[:, n2 * QC : (n2 + 1) * QC], ps[:])
        nc.sync.dma_start(out[s * P : (s + 1) * P, :], ost[:])

    for _ in norm_tail_gen:  # drain hp3's normalization
        pass
    for s in range(SEQ_T):
        proj_chunk(s)
    ostage_pool.release()
    wp_pool.release()
    dcol_pool.release()
    oT_pool.release()

    xt_pool.release()
    psum_acc.release()
    psum_mm.release()
    const_pool.release()


_NC_CACHE = None


def _get_program():
    global _NC_CACHE
    if _NC_CACHE is None:
        _NC_CACHE = _build_core_program()
    return _NC_CACHE


def _make_in_maps(x, w_qkv, w_proj):
    x = np.ascontiguousarray(np.asarray(x, dtype=np.float32))
    w_qkv = np.ascontiguousarray(np.asarray(w_qkv, dtype=np.float32))
    w_proj = np.ascontiguousarray(np.asarray(w_proj, dtype=np.float32))
    in_maps = []
    for core in range(N_CORES):
        b, g = core // 2, core % 2
        cs = slice(g * COLS, (g + 1) * COLS)
        in_maps.append(
            {
                "x": np.ascontiguousarray(x[b]),
                "wq": np.ascontiguousarray(w_qkv[:, 0 * DM : 1 * DM][:, cs]),
                "wk": np.ascontiguousarray(w_qkv[:, 1 * DM : 2 * DM][:, cs]),
                "wv": np.ascontiguousarray(w_qkv[:, 2 * DM : 3 * DM][:, cs]),
                "wp": np.ascontiguousarray(w_proj[cs, :]),
            }
        )
    return in_maps


def run_on_hw(x, w_qkv, w_proj, trace=False, **kwargs):
    """Run the SPMD program on 8 cores; returns (full_output, BassKernelResults)."""
    nc = _get_program()
    in_maps = _make_in_maps(x, w_qkv, w_proj)
    res = run_bass_kernel_spmd(
        nc, in_maps, list(range(N_CORES)), trace=trace, **kwargs
    )
    bs = 4
    outp = np.empty((bs, SEQ, DM), dtype=np.float32)
    for b in range(bs):
        outp[b] = res.results[2 * b]["out"] + res.results[2 * b + 1]["out"]
    return outp, res


def kernel(x, w_qkv, w_proj):
    outp, _ = run_on_hw(x, w_qkv, w_proj, trace=False)
    return outp



# revision 18
# speedup vs baseline: 1.2295x; 1.2295x over previous
"""Causal multi-head self-attention on 8 trn2 NeuronCores.

Sharding: 8 cores = 4 batch x 2 head-groups. Core i handles batch i//2 and
heads (i%2)*8 .. (i%2)*8+8 (8 of 16 heads, 512 of 1024 d_model columns).
Each core computes a full (2048, 1024) partial output (its head group pushed
through its w_proj row-slice); the host sums the two partials per batch
element (the tensor-parallel all-reduce done host-side).

All inputs are converted to bf16 on the host; x is host-transposed so x^T
DMAs straight into SBUF (no PE transposes, no staging casts). Per-core
dataflow, everything in transposed layout:
  Q^T, K^T   : w_q/w_k stationary, x^T moving  -> [cols, seq] bf16
  V_aug      : x^T stationary, w_v moving      -> natural [seq, cols] bf16
               + a ones-column per head so the softmax denominator rides the
               AV matmul as output row 64
  S^T        : K^T stationary, Q^T moving; the two heads of a pair go to PE
               row groups 0-63 / 64-127 (K=64) and run concurrently; two
               k-tiles of S land in one 2-bank PSUM tile [128, 1024]
  P^T        : one exp per (head, kt-pair) over the 2-bank tile on ScalarE
               (no max-subtraction: |S|*rsqrt < ~10); causal = skip k>q
               chunks, zero invalid prefixes, 0/1 mask mul on diag blocks
  O^T_aug    : V_aug stationary, P^T moving, accumulated over k-tiles in PSUM
  normalize  : denominators for (h0, h1) of a q-chunk drain to two SBUF
               partitions; 1/d via one reciprocal_approx_fast (DVE), then a
               single K=2 matmul against a 0/1 selector broadcasts both
               reciprocal rows over the pair's 128 partitions; one DVE mul
  out        : O^T stationary, w_proj rows moving -> natural [seq, 1024] f32

Schedule: one strictly-ordered PE chain (dep edges pin every matmul). Per
kt-pair step: 4 S matmuls (pairs packed via row groups), filler units
(next pair's QKV chunks, V tail, normalizes, and for the last head pair the
projection s-tiles), then the previous step's 4 AV matmuls. ScalarE chews
exp one step behind the S matmuls; AV trails exp by a step.
"""

import numpy as np
import ml_dtypes

import concourse.bass as bass
import concourse.mybir as mybir
import concourse.tile as tile
from concourse import bacc
from concourse.bass_utils import run_bass_kernel_spmd
from concourse.masks import make_upper_triangular
from concourse.tile_rust import add_dep_helper
from collections import deque

F32 = mybir.dt.float32
BF16 = mybir.dt.bfloat16
AF = mybir.ActivationFunctionType

SEQ = 2048
DM = 1024
COLS = 512          # head-cols per core (8 heads x 64)
HD = 64
P = 128
N_CORES = 8
RSQRT = 0.125       # 1/sqrt(64)

SEQ_T = SEQ // P    # 16 seq tiles
DM_T = DM // P      # 8 d_model tiles
QC = 512            # q-chunk (PSUM free size)
N_QC = SEQ // QC    # 4 q chunks
NP = 4              # head pairs per core


def _build_core_program():
    nc = bacc.Bacc(
        "TRN2", target_bir_lowering=False, debug=False, num_devices=N_CORES
    )
    xT = nc.dram_tensor("xT", [DM, SEQ], BF16, kind="ExternalInput").ap()
    sel = nc.dram_tensor("sel", [33, P], F32, kind="ExternalInput").ap()
    wq = nc.dram_tensor("wq", [DM, COLS], BF16, kind="ExternalInput").ap()
    wk = nc.dram_tensor("wk", [DM, COLS], BF16, kind="ExternalInput").ap()
    wv = nc.dram_tensor("wv", [DM, COLS], BF16, kind="ExternalInput").ap()
    wp = nc.dram_tensor("wp", [COLS, DM], BF16, kind="ExternalInput").ap()
    out = nc.dram_tensor("out", [SEQ, DM], F32, kind="ExternalOutput").ap()

    with tile.TileContext(nc) as tc:
        _emit(tc, xT, sel, wq, wk, wv, wp, out)
    nc.compile()
    return nc


def _emit(tc, xT, sel, wq, wk, wv, wp, out):
    nc = tc.nc

    # strict PE order: every matmul chains onto the previous one (order-only
    # edge, no semaphore) so the scheduler cannot interpose PE work between
    # an S row-group pair, which would break their concurrent execution
    chain = [None]

    def pin(mm):
        if chain[0] is not None:
            add_dep_helper(mm.ins, chain[0], sync=False, reason="pe-chain")
        chain[0] = mm.ins

    # --- pools ------------------------------------------------------------
    const_pool = tc.alloc_tile_pool(name="const", bufs=1)
    ps_s = tc.alloc_tile_pool(name="ps_s", bufs=2, space="PSUM")      # 4 banks
    psum_mm = tc.alloc_tile_pool(name="psum_mm", bufs=2, space="PSUM")  # 2
    psum_acc = tc.alloc_tile_pool(name="psum_acc", bufs=2, space="PSUM")  # 2

    # --- constants --------------------------------------------------------
    mask01 = const_pool.tile([P, P], BF16, tag="mask01")
    # 1.0 where free-idx (q) >= partition-idx (k), else 0 — causal in S^T
    make_upper_triangular(nc, mask01[:], val=1.0, diag=True)
    cstage = const_pool.tile([P, QC], F32, tag="cstage")
    nc.vector.memset(cstage[:], 0.0)
    zeros512 = const_pool.tile([P, QC], BF16, tag="zeros512")
    nc.vector.tensor_copy(zeros512[:], cstage[:])
    nc.vector.memset(cstage[:, 0:8], 1.0)
    ones8 = const_pool.tile([P, 8], BF16, tag="ones8")
    nc.vector.tensor_copy(ones8[:], cstage[:, 0:8])
    # 0/1 selector for the reciprocal broadcast (host-built): contraction row
    # 0 carries h0's reciprocal to out partitions 0-63, row 32 carries h1's
    # to 64-127
    sel33 = const_pool.tile([33, P], F32, tag="sel33")
    nc.sync.dma_start(sel33[:], sel[:, :])

    # --- persistent SBUF --------------------------------------------------
    xt_pool = tc.alloc_tile_pool(name="xt", bufs=1)
    xt = xt_pool.tile([P, DM_T * SEQ], BF16, tag="xt")  # x^T, d-tile major
    wv_pool = tc.alloc_tile_pool(name="wv", bufs=1)
    wv_sb = wv_pool.tile([P, DM_T * COLS], BF16, tag="wv_sb")
    vaug_pool = tc.alloc_tile_pool(name="vaug", bufs=1)
    vaug = vaug_pool.tile([P, SEQ_T * 8 * (HD + 1)], BF16, tag="vaug")
    vaug_v = vaug[:].rearrange("p (s h e) -> p s h e", s=SEQ_T, h=8)
    oT_pool = tc.alloc_tile_pool(name="oT", bufs=1)
    oT = oT_pool.tile([P, NP * SEQ], BF16, tag="oT")
    wp_pool = tc.alloc_tile_pool(name="wp", bufs=1)
    wp_sb = wp_pool.tile([P, 4 * DM], BF16, tag="wp_sb")

    qk_pool = tc.alloc_tile_pool(name="qk", bufs=2)
    wqk_pool = tc.alloc_tile_pool(name="wqk", bufs=2)
    pt_pool = tc.alloc_tile_pool(name="pt", bufs=6)
    dcol_pool = tc.alloc_tile_pool(name="dcol", bufs=2)
    rcol_pool = tc.alloc_tile_pool(name="rcol", bufs=2)
    ostage_pool = tc.alloc_tile_pool(name="ostage", bufs=3)

    # --- input DMAs (bf16, direct into final layout) ----------------------
    for d in range(DM_T):
        nc.sync.dma_start(
            xt[:, d * SEQ : (d + 1) * SEQ], xT[d * P : (d + 1) * P, :]
        )
        nc.sync.dma_start(
            wv_sb[:, d * COLS : (d + 1) * COLS], wv[d * P : (d + 1) * P, :]
        )
    for c in range(4):
        nc.sync.dma_start(
            wp_sb[:, c * DM : (c + 1) * DM], wp[c * P : (c + 1) * P, :]
        )

    # ===== V (natural layout) + ones columns ==============================
    def v_mms(s):
        ps = psum_mm.tile([P, QC], F32, tag="mm", name=f"vps_{s}")
        for d in range(DM_T):
            mm = nc.tensor.matmul(
                ps[:],
                xt[:, d * SEQ + s * P : d * SEQ + (s + 1) * P],
                wv_sb[:, d * COLS : (d + 1) * COLS],
                start=(d == 0),
                stop=(d == DM_T - 1),
            )
            pin(mm)
            if d % 2 == 1 and d < DM_T - 1:
                yield
        nc.vector.tensor_copy(
            vaug_v[:, s, :, 0:HD], ps[:].rearrange("p (h e) -> p h e", h=8)
        )
        nc.vector.tensor_copy(
            vaug_v[:, s, :, HD : HD + 1],
            ones8[:].rearrange("p (a b) -> p a b", b=1),
        )
        yield

    for s in range(12):  # first 12 seq-tiles upfront; rest as fillers
        for _ in v_mms(s):
            pass

    # ===== Q^T / K^T pair machinery =======================================
    def qk_pair_dma(hp):
        wq_sb = wqk_pool.tile([P, DM_T * P], BF16, tag="wq_sb", name=f"wq{hp}")
        wk_sb = wqk_pool.tile([P, DM_T * P], BF16, tag="wk_sb", name=f"wk{hp}")
        nc.sync.dma_start(
            wq_sb[:].rearrange("p (d c) -> p d c", d=DM_T),
            wq[:, hp * P : (hp + 1) * P].rearrange("(d p) c -> p d c", p=P),
        )
        nc.sync.dma_start(
            wk_sb[:].rearrange("p (d c) -> p d c", d=DM_T),
            wk[:, hp * P : (hp + 1) * P].rearrange("(d p) c -> p d c", p=P),
        )
        qT = qk_pool.tile([P, SEQ], BF16, tag="qT", name=f"qT{hp}")
        kT = qk_pool.tile([P, SEQ], BF16, tag="kT", name=f"kT{hp}")
        return dict(wq_sb=wq_sb, wk_sb=wk_sb, qT=qT, kT=kT, hp=hp)

    def qk_chunks(pair):
        # k chunk n before q chunk n; chunk n is needed when attention
        # reaches q-chunk n of this pair
        for n in range(N_QC):
            for wsb, dst in (
                (pair["wk_sb"], pair["kT"]),
                (pair["wq_sb"], pair["qT"]),
            ):
                ps = psum_mm.tile(
                    [P, QC], F32, tag="mm", name=f"qkps_{pair['hp']}_{n}"
                )
                for d in range(DM_T):
                    mm = nc.tensor.matmul(
                        ps[:],
                        wsb[:, d * P : (d + 1) * P],
                        xt[:, d * SEQ + n * QC : d * SEQ + (n + 1) * QC],
                        start=(d == 0),
                        stop=(d == DM_T - 1),
                    )
                    pin(mm)
                    if d % 2 == 1 and d < DM_T - 1:
                        yield
                nc.vector.tensor_copy(dst[:, n * QC : (n + 1) * QC], ps[:])
                yield

    # ===== filler queue ===================================================
    fillers = deque()  # (key, generator) — advanced one unit at a time

    def advance_filler():
        while fillers:
            key, gen = fillers[0]
            try:
                next(gen)
                return True
            except StopIteration:
                fillers.popleft()
        return False

    def finish_filler(want_key):
        for key, gen in list(fillers):
            if key == want_key:
                for _ in gen:
                    pass
                fillers.remove((key, gen))

    def v_rest_gen():
        for s in range(12, SEQ_T):
            for _ in v_mms(s):
                yield

    def norm_gen(hp, qc, dcol):
        # 1/d on DVE (no ScalarE table thrash): rows 0/32 hold the two
        # heads' denominators; one K=33 matmul against the 0/1 selector
        # broadcasts both reciprocal rows over the pair's 128 partitions.
        # Unit 1 is DVE-only so the pinned PE chain never waits on it.
        rcol = rcol_pool.tile([P, QC], F32, tag="rcol", name=f"rc_{hp}_{qc}")
        nc.vector.reciprocal_approx_fast(rcol[0:33, :], dcol[0:33, :])
        yield
        rc = psum_mm.tile([P, QC], F32, tag="mm", name=f"rcb_{hp}_{qc}")
        mm = nc.tensor.matmul(
            rc[:], sel33[0:33, :], rcol[0:33, :], start=True, stop=True
        )
        pin(mm)
        sl = oT[:, hp * SEQ + qc * QC : hp * SEQ + (qc + 1) * QC]
        nc.vector.tensor_mul(sl, sl, rc[:])
        yield

    def proj_gen(s):
        ost = ostage_pool.tile([P, DM], F32, tag="ost", name=f"ost_{s}")
        for n2 in range(2):
            ps = psum_mm.tile([P, QC], F32, tag="mm", name=f"pps_{s}_{n2}")
            for c in range(4):
                mm = nc.tensor.matmul(
                    ps[:],
                    oT[:, c * SEQ + s * P : c * SEQ + (s + 1) * P],
                    wp_sb[:, c * DM + n2 * QC : c * DM + (n2 + 1) * QC],
                    start=(c == 0),
                    stop=(c == 3),
                )
                pin(mm)
                if c == 1:
                    yield
            nc.vector.tensor_copy(ost[:, n2 * QC : (n2 + 1) * QC], ps[:])
            yield
        nc.sync.dma_start(out[s * P : (s + 1) * P, :], ost[:])

    # ===== attention ======================================================
    av_fifo = deque()
    po_cur = {}
    dcol_cur = [None]

    def emit_av_step():
        if not av_fifo:
            return
        rec = av_fifo.popleft()
        hp, qc, nkt, pts = rec["hp"], rec["qc"], rec["nkt"], rec["pts"]
        for idx, kt in enumerate(rec["kts"]):
            for hh in range(2):
                if kt == 0:
                    po_cur[hh] = psum_acc.tile(
                        [P, QC], F32, tag="po", name=f"po_{hp}_{qc}_{hh}"
                    )
                po = po_cur[hh]
                mm = nc.tensor.matmul(
                    po[0 : HD + 1, :],
                    vaug_v[:, kt, 2 * hp + hh, :],
                    pts[hh][:, idx * QC : (idx + 1) * QC],
                    start=(kt == 0),
                    stop=(kt == nkt - 1),
                )
                pin(mm)
                if kt == nkt - 1:
                    if hh == 0:
                        dcol_cur[0] = dcol_pool.tile(
                            [P, QC], F32, tag="dcol", name=f"dc_{hp}_{qc}"
                        )
                        # rows 1-31 must be finite for the [0:33] reciprocal
                        nc.vector.memset(dcol_cur[0][0:HD, :], 1.0)
                    dcol = dcol_cur[0]
                    nc.vector.tensor_copy(
                        dcol[32 * hh : 32 * hh + 1, :], po[HD : HD + 1, :]
                    )
                    nc.vector.tensor_copy(
                        oT[
                            hh * HD : (hh + 1) * HD,
                            hp * SEQ + qc * QC : hp * SEQ + (qc + 1) * QC,
                        ],
                        po[0:HD, :],
                    )
                    if hh == 1:
                        fillers.appendleft(
                            ("norm", norm_gen(hp, qc, dcol))
                        )
                        if hp == NP - 1:
                            for s in range(4 * qc, 4 * qc + 4):
                                fillers.append(("proj", proj_gen(s)))

    pair_cur = qk_pair_dma(0)
    for _ in qk_chunks(pair_cur):  # pair 0 fully upfront
        pass

    budget = {0: 4, 1: 3, 2: 3, 3: 3}
    for hp in range(NP):
        if hp == 0:
            fillers.append(("vrest", v_rest_gen()))
        if hp < NP - 1:
            pair_nxt = qk_pair_dma(hp + 1)
            fillers.append((("qk", hp + 1), qk_chunks(pair_nxt)))
        qT, kT = pair_cur["qT"], pair_cur["kT"]
        for qc in range(N_QC):
            nkt = 4 * qc + 4
            for j in range(nkt // 2):
                k0 = 2 * j
                A = ps_s.tile([P, 2 * QC], F32, tag="s2",
                              name=f"sA_{hp}_{qc}_{j}")
                B = ps_s.tile([P, 2 * QC], F32, tag="s2",
                              name=f"sB_{hp}_{qc}_{j}")
                tiles = {0: A, 1: B}
                for idx, kt in enumerate((k0, k0 + 1)):
                    for hh in range(2):
                        b = hh * HD
                        mm = nc.tensor.matmul(
                            tiles[hh][:, idx * QC : (idx + 1) * QC],
                            kT[b : b + HD, kt * P : (kt + 1) * P],
                            qT[b : b + HD, qc * QC : (qc + 1) * QC],
                            start=True,
                            stop=True,
                        )
                        pin(mm)
                pts = {}
                diag = k0 >= 4 * qc
                for hh in range(2):
                    pt = pt_pool.tile([P, 2 * QC], BF16, tag="pt",
                                      name=f"pt_{hp}_{qc}_{j}_{hh}")
                    pts[hh] = pt
                    if not diag:
                        nc.scalar.activation(
                            pt[:], tiles[hh][:], AF.Exp, scale=RSQRT
                        )
                    else:
                        r0 = k0 - 4 * qc
                        off0, off1 = r0 * P, (r0 + 1) * P
                        nc.scalar.activation(
                            pt[:, off0 : 2 * QC],
                            tiles[hh][:, off0 : 2 * QC],
                            AF.Exp,
                            scale=RSQRT,
                        )
                        if off0 > 0:
                            nc.vector.tensor_copy(
                                pt[:, 0:off0], zeros512[:, 0:off0]
                            )
                        nc.vector.tensor_copy(
                            pt[:, QC : QC + off1], zeros512[:, 0:off1]
                        )
                        nc.vector.tensor_mul(
                            pt[:, off0 : off0 + P],
                            pt[:, off0 : off0 + P],
                            mask01[:],
                        )
                        nc.vector.tensor_mul(
                            pt[:, QC + off1 : QC + off1 + P],
                            pt[:, QC + off1 : QC + off1 + P],
                            mask01[:],
                        )
                av_fifo.append(
                    dict(hp=hp, qc=qc, nkt=nkt, pts=pts, kts=(k0, k0 + 1))
                )
                for _ in range(budget[hp]):
                    advance_filler()
                if len(av_fifo) >= 2:  # AV trails S by 2 steps (exp latency)
                    emit_av_step()
        if hp < NP - 1:
            finish_filler(("qk", hp + 1))  # next pair's QK must be complete
            pair_cur = pair_nxt
    while av_fifo:  # drain trailing AV steps
        emit_av_step()
    while advance_filler():  # norms + remaining proj
        pass

    ostage_pool.release()
    rcol_pool.release()
    dcol_pool.release()
    pt_pool.release()
    wqk_pool.release()
    qk_pool.release()
    wp_pool.release()
    oT_pool.release()
    vaug_pool.release()
    wv_pool.release()
    xt_pool.release()
    psum_acc.release()
    psum_mm.release()
    ps_s.release()
    const_pool.release()


_NC_CACHE = None


def _get_program():
    global _NC_CACHE
    if _NC_CACHE is None:
        _NC_CACHE = _build_core_program()
    return _NC_CACHE


BF = ml_dtypes.bfloat16


def _make_in_maps(x, w_qkv, w_proj):
    x = np.asarray(x, dtype=np.float32)
    w_qkv = np.asarray(w_qkv, dtype=np.float32)
    w_proj = np.asarray(w_proj, dtype=np.float32)
    in_maps = []
    for core in range(N_CORES):
        b, g = core // 2, core % 2
        cs = slice(g * COLS, (g + 1) * COLS)
        sel33 = np.zeros((33, P), dtype=np.float32)
        sel33[0, 0:HD] = 1.0
        sel33[32, HD:P] = 1.0
        in_maps.append(
            {
                "xT": np.ascontiguousarray(x[b].T).astype(BF),
                "sel": sel33,
                "wq": np.ascontiguousarray(
                    w_qkv[:, 0 * DM : 1 * DM][:, cs]
                ).astype(BF),
                "wk": np.ascontiguousarray(
                    w_qkv[:, 1 * DM : 2 * DM][:, cs]
                ).astype(BF),
                "wv": np.ascontiguousarray(
                    w_qkv[:, 2 * DM : 3 * DM][:, cs]
                ).astype(BF),
                "wp": np.ascontiguousarray(w_proj[cs, :]).astype(BF),
            }
        )
    return in_maps


def run_on_hw(x, w_qkv, w_proj, trace=False, **kwargs):
    """Run the SPMD program on 8 cores; returns (full_output, BassKernelResults)."""
    nc = _get_program()
    in_maps = _make_in_maps(x, w_qkv, w_proj)
    res = run_bass_kernel_spmd(
        nc, in_maps, list(range(N_CORES)), trace=trace, **kwargs
    )
    bs = 4
    outp = np.empty((bs, SEQ, DM), dtype=np.float32)
    for b in range(bs):
        outp[b] = res.results[2 * b]["out"] + res.results[2 * b + 1]["out"]
    return outp, res


def kernel(x, w_qkv, w_proj):
    outp, _ = run_on_hw(x, w_qkv, w_proj, trace=False)
    return outp
